# revision 9
# baseline (speedup 1.0000x reference)
"""HRR binding self-attention kernel for 8 trn2 NeuronCores (v2).

Math: out = irfft(c * rfft(x) * cumsum_s(rfft(x))) @ w_out.T  with c = queries*keyvalues.
rfft is linear, so cumsum commutes with it: one forward DFT of x; the causal
prefix sum runs in the frequency domain.  irfft is also linear, so it FUSES into
the output Linear: out = qv^T @ GW with GW = (c * Gf) @ w_out.T precomputed on
host (the c filter rides along for free since complex scalars commute).

Sharding: 8 shards = (batch b in 0..3) x (seq half h in 0..1), 2048 tokens each.
The h=1 shards get the first half's contribution as an initial carry, computed
on host as rfft(x[b, :2048].sum(0)) (negligible).

Packed real spectrum (2048 rows): rows 0..1024 = Re[0..1024], rows 1025..2047 =
Im[1..1023].  Row 1024 (Nyquist) rides in the Im-block's first slot (chunk 8,
partition 0); complex multiplies pair chunk c with chunk 8+c on equal
partitions, with a 2-row fixup for the DC/Nyquist slots.

Per-core pipeline, one pass over 8 slabs of 256 tokens (matmuls bf16, f32 PSUM):
  - transposed DFT: CS chunk stationary, x-slab moving -> freq-major spectrum
    [pk, tok] straight into PSUM (no token-major intermediate, no transpose);
  - Q copied to SBUF (ACT), then tensor_tensor_scan runs the causal cumsum
    in-place in PSUM (f32 state, per-partition carry chained across slabs);
  - complex multiply per chunk-pair (c, 8+c) on DVE -> qv bf16;
  - output matmul qv^T (stationary) @ GW (moving) -> out rows, f32.
Emission interleaves slab s's DFT with slab s-1's output matmul so the PE
never idles.
"""

import sys

sys.path.insert(0, "/opt/trn_rl_repo")

import numpy as np
import ml_dtypes

import concourse.bass as bass
import concourse.bacc as bacc
import concourse.mybir as mybir
from concourse.tile import TileContext
from concourse.bass_utils import run_bass_kernel_spmd

BF16 = mybir.dt.bfloat16
F32 = mybir.dt.float32
ADD = mybir.AluOpType.add
BYP = mybir.AluOpType.bypass

P = 128
D = 2048  # model dims
T = 2048  # tokens per shard
ND = D // P  # 16 d-chunks
NPF = 16  # packed-frequency chunks
TSB = 512  # tokens per slab
NSLAB = T // TSB  # 4
NB = 4  # batch
NS = 4096  # full seq

bf16 = ml_dtypes.bfloat16

_CACHE = {}


def _build_nc(reps: int = 1):
    nc = bacc.Bacc("TRN2", target_bir_lowering=False, debug=False, num_devices=8)
    xT = nc.dram_tensor("xT", [NSLAB, P, ND, TSB], BF16, kind="ExternalInput")
    CS2 = nc.dram_tensor("CS2", [NPF, P, ND, P], BF16, kind="ExternalInput")
    GW = nc.dram_tensor("GW", [P, NPF, D], BF16, kind="ExternalInput")
    C0 = nc.dram_tensor("C0", [P, NPF], F32, kind="ExternalInput")
    out = nc.dram_tensor("out", [T, D], F32, kind="ExternalOutput")

    with TileContext(nc) as tc:
        with tc.tile_pool(name="misc", bufs=1) as misc:
            c0_sb = misc.tile([P, NPF], F32)
            nc.sync.dma_start(c0_sb[:], C0[:])

            import contextlib

            loop_ctx = (
                tc.For_i(0, reps, 1, staggered_reset=True)
                if reps > 1
                else contextlib.nullcontext()
            )
            with loop_ctx:
                _body(nc, tc, c0_sb, CS2, GW, xT, out)
    nc.finalize()
    return nc


def _body(nc, tc, c0_sb, CS2, GW, xT, out):
    with (
        tc.tile_pool(name="wts", bufs=1) as wpool,
        tc.tile_pool(name="xt", bufs=2) as xpool,
        tc.tile_pool(name="qsb", bufs=3) as qpool,
        tc.tile_pool(name="qv", bufs=2) as qvpool,
        tc.tile_pool(name="carry", bufs=2) as cpool,
        tc.tile_pool(name="tmp", bufs=1) as tpool,
        tc.tile_pool(name="osb", bufs=2) as opool,
        tc.tile_pool(name="psD", bufs=4, space="PSUM") as psD,
        tc.tile_pool(name="psC", bufs=2, space="PSUM") as psC,
    ):
        cs_sb = wpool.tile([P, NPF, ND, P], BF16)
        for pf in range(NPF):
            nc.sync.dma_start(cs_sb[:, pf], CS2[pf])
        gw_sb = wpool.tile([P, NPF, D], BF16)
        for pf in range(NPF):
            nc.sync.dma_start(gw_sb[:, pf, :], GW[:, pf, :])

        carry_prev = None
        qv_prev = None
        for s in range(NSLAB + 1):
            if s < NSLAB:
                xt = xpool.tile([P, ND, TSB], BF16, tag="xt")
                for q in range(4):
                    nc.sync.dma_start(xt[:, 4 * q : 4 * q + 4, :], xT[s, :, 4 * q : 4 * q + 4, :])
                qv = qvpool.tile([P, NPF, TSB], BF16, tag="qv")
                carry_sb = cpool.tile([P, NPF], F32, tag="carry")
                Qp0 = None
                for c in range(8):
                    Qp = qpool.tile([P, 2, TSB], BF16, tag="Q")
                    if c == 0:
                        Qp0 = Qp
                    psts = {}
                    for h, pf in enumerate((c, 8 + c)):
                        pst = psD.tile([P, TSB], F32, tag="psD")
                        for dc in range(ND):
                            nc.tensor.matmul(
                                pst[:],
                                cs_sb[:, pf, dc, :],
                                xt[:, dc, :],
                                start=(dc == 0),
                                stop=(dc == ND - 1),
                            )
                        nc.scalar.copy(Qp[:, h, :], pst[:])
                        init = (
                            c0_sb[:, pf : pf + 1]
                            if s == 0
                            else carry_prev[:, pf : pf + 1]
                        )
                        # op1=bypass: state = data0 + state; data1 ignored
                        nc.vector.tensor_tensor_scan(
                            pst[:], pst[:], Qp[:, h, :], init, ADD, BYP
                        )
                        nc.scalar.copy(carry_sb[:, pf : pf + 1], pst[:, TSB - 1 : TSB])
                        psts[h] = pst
                    SR, SI = psts[0], psts[1]
                    QR, QI = Qp[:, 0, :], Qp[:, 1, :]
                    t1 = tpool.tile([P, TSB], F32, tag="t1")
                    t2 = tpool.tile([P, TSB], F32, tag="t2")
                    nc.vector.tensor_mul(t1[:], QR, SR[:])
                    nc.vector.tensor_mul(t2[:], QI, SI[:])
                    nc.vector.tensor_sub(qv[:, c, :], t1[:], t2[:])
                    t3 = tpool.tile([P, TSB], F32, tag="t1")
                    t4 = tpool.tile([P, TSB], F32, tag="t2")
                    nc.vector.tensor_mul(t3[:], QR, SI[:])
                    nc.vector.tensor_mul(t4[:], QI, SR[:])
                    nc.vector.tensor_add(qv[:, 8 + c, :], t3[:], t4[:])
                    if c == 0:
                        # DC (chunk 0 row 0) and Nyquist (chunk 8 row 0): purely real
                        nc.vector.tensor_mul(qv[0:1, 0, :], Qp0[0:1, 0, :], SR[0:1, :])
                        nc.vector.tensor_mul(qv[0:1, 8, :], Qp0[0:1, 1, :], SI[0:1, :])
                carry_prev = carry_sb

            if s > 0:
                for tb in range(TSB // P):
                    for e in range(4):
                        psc = psC.tile([P, 512], F32, tag="psC")
                        for pf in range(NPF):
                            nc.tensor.matmul(
                                psc[:],
                                qv_prev[:, pf, tb * P : (tb + 1) * P],
                                gw_sb[:, pf, e * 512 : (e + 1) * 512],
                                start=(pf == 0),
                                stop=(pf == NPF - 1),
                            )
                        osb = opool.tile([P, 512], F32, tag="osb")
                        if e % 2 == 0:
                            nc.scalar.copy(osb[:], psc[:])
                        else:
                            nc.vector.tensor_copy(osb[:], psc[:])
                        r0 = (s - 1) * TSB + tb * P
                        nc.sync.dma_start(
                            out[r0 : r0 + P, e * 512 : (e + 1) * 512], osb[:]
                        )
            if s < NSLAB:
                qv_prev = qv


def _chunked(m):
    """[rows, cols] -> [P, rows//P, cols] with row r at [r % P, r // P]."""
    r, c = m.shape
    return np.ascontiguousarray(m.reshape(r // P, P, c).transpose(1, 0, 2))


def _pack_spec(re, im):
    """re[1025], im[1025] -> packed [2048]: re[0..1024] then im[1..1023]."""
    return np.concatenate([re, im[1:1024]])


def _constants():
    if "consts" in _CACHE:
        return _CACHE["consts"]
    d = np.arange(D, dtype=np.float64)
    f = np.arange(D // 2 + 1, dtype=np.float64)
    ang = 2.0 * np.pi / D * np.outer(d, f)  # [D, 1025]
    cos, sin = np.cos(ang), np.sin(ang)
    CSf = np.concatenate([cos, -sin[:, 1:1024]], axis=1)  # [D, 2048] packed fwd
    alpha = np.full(1025, 2.0)
    alpha[0] = alpha[1024] = 1.0
    Gf = np.concatenate(
        [(alpha[:, None] * cos.T) / D, (-2.0 * sin[:, 1:1024].T) / D], axis=0
    )  # [2048 packed, D]
    # CS2[pf, p, dc, j] = CSf[128*dc + p, 128*pf + j]
    CS2 = np.ascontiguousarray(
        CSf.reshape(ND, P, NPF, P).transpose(2, 1, 0, 3)
    ).astype(np.float32)
    consts = {"CS2": CS2.astype(bf16), "Gf": Gf}
    _CACHE["consts"] = consts
    return consts


def kernel(x, queries, keyvalues, w_out):
    x = np.asarray(x, dtype=np.float32)
    queries = np.asarray(queries, dtype=np.float32)
    keyvalues = np.asarray(keyvalues, dtype=np.float32)
    w_out = np.asarray(w_out, dtype=np.float32)

    if "nc" not in _CACHE:
        _CACHE["nc"] = _build_nc()
    nc = _CACHE["nc"]
    consts = _constants()

    c = (queries * keyvalues).reshape(-1)  # [1025]
    c_packed = _pack_spec(c, c)  # [2048]
    GWf = (c_packed[:, None] * consts["Gf"]).astype(np.float32) @ w_out.T
    GWc = _chunked(GWf.astype(np.float32)).astype(bf16)  # [P, NPF, D]

    in_maps = []
    shards = []
    for b in range(NB):
        for h in range(2):
            shards.append((b, h))
            xs = x[b, h * T : (h + 1) * T]  # [T, D]
            xT3 = _chunked(np.ascontiguousarray(xs.T))  # [P, ND, T]
            xTc = np.ascontiguousarray(
                xT3.reshape(P, ND, NSLAB, TSB).transpose(2, 0, 1, 3)
            ).astype(bf16)
            if h == 0:
                c0 = np.zeros((P, NPF), np.float32)
            else:
                F = np.fft.rfft(x[b, :T].sum(axis=0).astype(np.float64))
                c0 = _chunked(
                    _pack_spec(F.real, F.imag).astype(np.float32)[:, None]
                )[:, :, 0]
            in_maps.append(
                {
                    "xT": xTc,
                    "CS2": consts["CS2"],
                    "GW": GWc,
                    "C0": np.ascontiguousarray(c0),
                }
            )

    global _LAST_IN_MAPS
    _LAST_IN_MAPS = in_maps
    res = run_bass_kernel_spmd(nc, in_maps, core_ids=list(range(8)))
    y = np.empty((NB, NS, D), np.float32)
    for i, (b, h) in enumerate(shards):
        y[b, h * T : (h + 1) * T] = res.results[i]["out"]
    return y
